# revision 6
# baseline (speedup 1.0000x reference)
"""DKD keypoint detection on 8 TRN2 NeuronCores.

Pure data parallel: 1 image per core.
Device: exact simple_nms on an int16-quantized score field.
  Scores are multiples of 2^-23 in [0,1). v = 30000 - (1-s)*2^23, clamped
  below at BG for non-candidates (s < T0 ~ 0.995). Comparisons among
  candidates are exact; background ordering below candidates is preserved
  or collapsed, which is provably equivalent for the final top-4096
  (T0 is far below the 4096th surviving score on any plausible input;
  survivor count is asserted host-side). 3-round NMS (maxpool / dilate /
  equality) runs densely in int16 with an even/odd column parity split.
  Output: post-NMS masked field (int16 planes).
Host: extract survivors (>4096 guaranteed), exact top-4096 ordering by
  (score desc, index asc) replicating jax.lax.top_k tie-breaks, then the
  tiny 4096x25 softmax / dispersity / bilinear tail in float32.
"""
import numpy as np

import concourse.bass as bass
import concourse.bacc as bacc
import concourse.mybir as mybir
from concourse.tile import TileContext
from concourse.bass_utils import run_bass_kernel_spmd

dt = mybir.dt
Alu = mybir.AluOpType
Act = mybir.ActivationFunctionType

H = W = 1024
P = 128            # partitions
RPP = 8            # image rows per partition
HR = 2             # halo rows each side
TR = RPP + 2 * HR  # tile rows = 12
WE = W // 2        # 512 per parity
WP = WE + 4        # 2 pad cols each side (interior at [2, 514))
BG = -28800        # background / suppressed value
M0 = 41944         # (1 - T0) * 2^23  -> T0 = 1 - 41944*2^-23 ~= 0.995
TOP_K = 4096
RADIUS = 2
KSIZE = 5
KK = 25
TEMPERATURE = 0.1

_CACHED = {}

LO, HI = 2, 514            # interior column range
RS, RE = HR, HR + RPP      # payload row range


def _maxpool5(nc, pool, fE, fO, outdt, taga, tagb):
    """5x5 separable max pool of parity pair (fE, fO) -> (oE, oO).

    Inputs need valid halo rows and pad cols. The x-stage runs on all TR
    rows (so y-stage needs no halo refresh); y-stage writes payload rows.
    """
    i16 = dt.int16
    mx = pool.tile([P, TR, WP], i16, tag="mp_mx")
    nc.vector.tensor_tensor(out=mx[:, :, 1:515], in0=fE[:, :, 1:515],
                            in1=fO[:, :, 1:515], op=Alu.max)
    xpE = pool.tile([P, TR, WP], i16, tag="mp_xpE")
    xpO = pool.tile([P, TR, WP], i16, tag="mp_xpO")
    # E col c (x = 2(c-2)): window = max(mx[c-1], mx[c], fE[c+1])
    nc.vector.tensor_tensor(out=xpE[:, :, LO:HI], in0=mx[:, :, LO - 1:HI - 1],
                            in1=mx[:, :, LO:HI], op=Alu.max)
    nc.vector.tensor_tensor(out=xpE[:, :, LO:HI], in0=xpE[:, :, LO:HI],
                            in1=fE[:, :, LO + 1:HI + 1], op=Alu.max)
    # O col c: window = max(mx[c], mx[c+1], fO[c-1])
    nc.vector.tensor_tensor(out=xpO[:, :, LO:HI], in0=mx[:, :, LO:HI],
                            in1=mx[:, :, LO + 1:HI + 1], op=Alu.max)
    nc.vector.tensor_tensor(out=xpO[:, :, LO:HI], in0=xpO[:, :, LO:HI],
                            in1=fO[:, :, LO - 1:HI - 1], op=Alu.max)

    outs = []
    for xp, tag in ((xpE, taga), (xpO, tagb)):
        m3 = pool.tile([P, TR, WP], i16, tag="mp_mx")   # reuse mx slot
        nc.vector.tensor_tensor(out=m3[:, 1:TR - 1, LO:HI], in0=xp[:, 0:TR - 2, LO:HI],
                                in1=xp[:, 2:TR, LO:HI], op=Alu.max)
        nc.vector.tensor_tensor(out=m3[:, 1:TR - 1, LO:HI], in0=m3[:, 1:TR - 1, LO:HI],
                                in1=xp[:, 1:TR - 1, LO:HI], op=Alu.max)
        o = pool.tile([P, TR, WP], outdt, tag=tag)
        nc.vector.tensor_tensor(out=o[:, RS:RE, LO:HI],
                                in0=m3[:, RS - 1:RE - 1, LO:HI],
                                in1=m3[:, RS + 1:RE + 1, LO:HI], op=Alu.max)
        outs.append(o)
    return outs


def _halo(nc, f, padcst):
    """Fill pad cols + halo rows of [P, TR, WP] field (payload valid).

    padcst: [P, HR * WP] tile holding the pad value (engine ops cannot start
    at partition 127, so outer halo rows are filled via DMA from it).
    """
    nc.vector.tensor_copy(f[:, RS:RE, 0:LO],
                          padcst[:, 0:1].to_broadcast([P, RPP, LO]))
    nc.vector.tensor_copy(f[:, RS:RE, HI:WP],
                          padcst[:, 0:1].to_broadcast([P, RPP, WP - HI]))
    nc.sync.dma_start(f[1:P, 0:HR, :], f[0:P - 1, RPP:RPP + HR, :])
    nc.sync.dma_start(f[0:P - 1, RE:TR, :], f[1:P, HR:2 * HR, :])
    nc.sync.dma_start(f[0:1, 0:HR, :],
                      padcst[0:1, :].rearrange("p (r w) -> p r w", r=HR))
    nc.sync.dma_start(f[P - 1:P, RE:TR, :],
                      padcst[0:1, :].rearrange("p (r w) -> p r w", r=HR))


def _build_nms_kernel():
    nc = bacc.Bacc(None, target_bir_lowering=False)
    img = nc.dram_tensor("img", [H, W], dt.float32, kind="ExternalInput")
    outE = nc.dram_tensor("outE", [H, WE], dt.int16, kind="ExternalOutput")
    outO = nc.dram_tensor("outO", [H, WE], dt.int16, kind="ExternalOutput")
    imgr = img.rearrange("(p r) w -> p r w", p=P)

    i16 = dt.int16
    with TileContext(nc) as tc:
        with tc.tile_pool(name="sb", bufs=1) as pool, \
             tc.tile_pool(name="ld", bufs=2) as ldp:
            bgc = pool.tile([P, HR * WP], i16, tag="bgc")
            nc.vector.memset(bgc[:], BG)
            zc = pool.tile([P, HR * WP], i16, tag="zc")
            nc.vector.memset(zc[:], 0)
            vE = pool.tile([P, TR, WP], i16, tag="vE")
            vO = pool.tile([P, TR, WP], i16, tag="vO")
            # ---- chunked load + v-compute ----
            for r in range(RPP):
                stage = ldp.tile([P, 1, W], dt.float32, tag="stage")
                nc.sync.dma_start(stage[:], imgr[:, r:r + 1, :])
                tf = ldp.tile([P, 1, W], dt.float32, tag="tf")
                nc.scalar.activation(tf[:], stage[:], Act.Copy,
                                     bias=float(30000 - (1 << 23)),
                                     scale=float(1 << 23))
                tv = tf[:].rearrange("p r (w two) -> p r two w", two=2)
                nc.vector.tensor_scalar(vE[:, RS + r:RS + r + 1, LO:HI],
                                        tv[:, :, 0, :], float(BG),
                                        scalar2=None, op0=Alu.max)
                nc.vector.tensor_scalar(vO[:, RS + r:RS + r + 1, LO:HI],
                                        tv[:, :, 1, :], float(BG),
                                        scalar2=None, op0=Alu.max)
            _halo(nc, vE, bgc)
            _halo(nc, vO, bgc)

            # ---- round 0 ----
            p1E, p1O = _maxpool5(nc, pool, vE, vO, i16, "pAE", "pAO")
            m0E = pool.tile([P, TR, WP], i16, tag="m0E")
            m0O = pool.tile([P, TR, WP], i16, tag="m0O")
            for v_, p_, m_ in ((vE, p1E, m0E), (vO, p1O, m0O)):
                nc.vector.tensor_tensor(out=m_[:, RS:RE, LO:HI],
                                        in0=v_[:, RS:RE, LO:HI],
                                        in1=p_[:, RS:RE, LO:HI], op=Alu.is_equal)
            _halo(nc, m0E, zc)
            _halo(nc, m0O, zc)

            # ---- round 1 ----
            s1E, s1O = _maxpool5(nc, pool, m0E, m0O, dt.int8, "sBE", "sBO")
            negc = pool.tile([P, 1], i16, tag="negc")
            nc.vector.memset(negc[:], BG)
            svE = pool.tile([P, TR, WP], i16, tag="svE")
            svO = pool.tile([P, TR, WP], i16, tag="svO")
            for v_, s_, sv_ in ((vE, s1E, svE), (vO, s1O, svO)):
                nc.vector.tensor_copy(sv_[:, RS:RE, LO:HI], v_[:, RS:RE, LO:HI])
                nc.vector.copy_predicated(sv_[:, RS:RE, LO:HI], s_[:, RS:RE, LO:HI],
                                          negc[:].to_broadcast([P, RPP, WE]))
            _halo(nc, svE, bgc)
            _halo(nc, svO, bgc)
            p3E, p3O = _maxpool5(nc, pool, svE, svO, i16, "pAE", "pAO")
            m1E = pool.tile([P, TR, WP], i16, tag="m1E")
            m1O = pool.tile([P, TR, WP], i16, tag="m1O")
            for sv_, p_, s_, m0_, m1_ in ((svE, p3E, s1E, m0E, m1E),
                                          (svO, p3O, s1O, m0O, m1O)):
                eq = pool.tile([P, TR, WP], dt.int8, tag="eq")
                nc.vector.tensor_tensor(out=eq[:, RS:RE, LO:HI],
                                        in0=sv_[:, RS:RE, LO:HI],
                                        in1=p_[:, RS:RE, LO:HI], op=Alu.is_equal)
                # new1 = eq & !supp1 ; mask1 = max(mask0, new1)
                nc.vector.scalar_tensor_tensor(out=eq[:, RS:RE, LO:HI],
                                               in0=s_[:, RS:RE, LO:HI], scalar=0.0,
                                               in1=eq[:, RS:RE, LO:HI],
                                               op0=Alu.is_equal, op1=Alu.mult)
                nc.vector.tensor_tensor(out=m1_[:, RS:RE, LO:HI],
                                        in0=m0_[:, RS:RE, LO:HI],
                                        in1=eq[:, RS:RE, LO:HI], op=Alu.max)
            _halo(nc, m1E, zc)
            _halo(nc, m1O, zc)

            # ---- round 2 ----
            s2E, s2O = _maxpool5(nc, pool, m1E, m1O, dt.int8, "sBE", "sBO")
            for v_, s_, sv_ in ((vE, s2E, svE), (vO, s2O, svO)):
                nc.vector.tensor_copy(sv_[:, RS:RE, LO:HI], v_[:, RS:RE, LO:HI])
                nc.vector.copy_predicated(sv_[:, RS:RE, LO:HI], s_[:, RS:RE, LO:HI],
                                          negc[:].to_broadcast([P, RPP, WE]))
            _halo(nc, svE, bgc)
            _halo(nc, svO, bgc)
            p5E, p5O = _maxpool5(nc, pool, svE, svO, i16, "pAE", "pAO")

            for v_, sv_, p_, s_, m1_, out_ in (
                    (vE, svE, p5E, s2E, m1E, outE),
                    (vO, svO, p5O, s2O, m1O, outO)):
                eq = pool.tile([P, TR, WP], dt.int8, tag="eq")
                nc.vector.tensor_tensor(out=eq[:, RS:RE, LO:HI],
                                        in0=sv_[:, RS:RE, LO:HI],
                                        in1=p_[:, RS:RE, LO:HI], op=Alu.is_equal)
                nc.vector.scalar_tensor_tensor(out=eq[:, RS:RE, LO:HI],
                                               in0=s_[:, RS:RE, LO:HI], scalar=0.0,
                                               in1=eq[:, RS:RE, LO:HI],
                                               op0=Alu.is_equal, op1=Alu.mult)
                nc.vector.tensor_tensor(out=eq[:, RS:RE, LO:HI],
                                        in0=m1_[:, RS:RE, LO:HI],
                                        in1=eq[:, RS:RE, LO:HI], op=Alu.max)
                svf = pool.tile([P, TR, WP], i16, tag="mp_xpE")  # reuse slot
                nc.vector.memset(svf[:, RS:RE, LO:HI], BG)
                nc.vector.copy_predicated(svf[:, RS:RE, LO:HI], eq[:, RS:RE, LO:HI],
                                          v_[:, RS:RE, LO:HI])
                nc.sync.dma_start(out_.rearrange("(p r) w -> p r w", p=P)[:, :, :],
                                  svf[:, RS:RE, LO:HI])
    nc.compile()
    return nc


def _numpy_nms(img):
    """Full-precision numpy replica of the reference NMS (fallback path)."""
    def maxpool(x):
        p = np.pad(x, RADIUS, mode="constant", constant_values=-np.inf)
        out = np.full_like(x, -np.inf)
        for dy in range(KSIZE):
            for dx in range(KSIZE):
                out = np.maximum(out, p[dy:dy + H, dx:dx + W])
        return out

    zeros = np.zeros_like(img)
    max_mask = img == maxpool(img)
    for _ in range(2):
        supp = maxpool(max_mask.astype(np.float32)) > 0
        ss = np.where(supp, zeros, img)
        new = ss == maxpool(ss)
        max_mask = max_mask | (new & ~supp)
    nms = np.where(max_mask, img, 0.0)
    nms[:2] = nms[-2:] = 0.0
    nms[:, :2] = nms[:, -2:] = 0.0
    flat = nms.reshape(-1)
    idx = np.argsort(-flat, kind="stable")[:TOP_K]
    return idx // W, idx % W


def _host_tail(img, vE, vO):
    """Exact top-4096 + keypoint refinement from the device NMS field."""
    if vE is None:
        ys, xs = _numpy_nms(img)
    else:
        sv = np.empty((H, W), np.int16)
        sv[:, 0::2] = vE
        sv[:, 1::2] = vO
        sv[:2] = BG; sv[-2:] = BG; sv[:, :2] = BG; sv[:, -2:] = BG
        ys, xs = np.nonzero(sv > BG)
        if len(ys) < TOP_K:
            # T0 margin insufficient for this input: recompute exactly on host.
            ys, xs = _numpy_nms(img)
        else:
            m = 30000 - sv[ys, xs].astype(np.int64)
            idx = ys.astype(np.int64) * W + xs
            order = np.lexsort((idx, m))[:TOP_K]
            ys, xs = ys[order], xs[order]

    padded = np.pad(img, ((RADIUS, RADIUS), (RADIUS, RADIUS)))
    patches = np.empty((TOP_K, KSIZE, KSIZE), np.float32)
    for dy in range(KSIZE):
        for dx in range(KSIZE):
            patches[:, dy, dx] = padded[ys + dy, xs + dx]
    patches = patches.reshape(TOP_K, KK)

    lin = np.linspace(-RADIUS, RADIUS, KSIZE, dtype=np.float32)
    gx, gy = np.meshgrid(lin, lin)
    hw_grid = np.stack([gx, gy], axis=-1).reshape(-1, 2).astype(np.float32)

    max_v = patches.max(axis=-1, keepdims=True)
    x_exp = np.exp((patches - max_v) / np.float32(TEMPERATURE), dtype=np.float32)
    x_exp_sum = x_exp.sum(axis=-1, keepdims=True, dtype=np.float32)
    xy_res = (x_exp @ hw_grid) / x_exp_sum
    diff = (hw_grid[None, :, :] - xy_res[:, None, :]) / np.float32(RADIUS)
    hw_dist2 = (diff * diff).sum(-1, dtype=np.float32)
    dispersity = (x_exp * hw_dist2).sum(-1, dtype=np.float32) / x_exp_sum[:, 0]

    kp_nms = np.stack([xs, ys], axis=-1).astype(np.float32)
    wh = np.array([W - 1, H - 1], np.float32)
    kxy = (kp_nms + xy_res) / wh * np.float32(2.0) - np.float32(1.0)

    px = ((kxy[:, 0] + 1.0) * 0.5 * (W - 1)).astype(np.float32)
    py = ((kxy[:, 1] + 1.0) * 0.5 * (H - 1)).astype(np.float32)
    x0 = np.floor(px); y0 = np.floor(py)
    wx1 = (px - x0).astype(np.float32); wx0 = np.float32(1.0) - wx1
    wy1 = (py - y0).astype(np.float32); wy0 = np.float32(1.0) - wy1

    def tap(xi, yi, wgt):
        valid = (xi >= 0) & (xi < W) & (yi >= 0) & (yi < H)
        xc = np.clip(xi, 0, W - 1).astype(np.int64)
        yc = np.clip(yi, 0, H - 1).astype(np.int64)
        return np.where(valid, img[yc, xc], 0.0).astype(np.float32) * wgt

    kpscore = (tap(x0, y0, wx0 * wy0) + tap(x0 + 1, y0, wx1 * wy0)
               + tap(x0, y0 + 1, wx0 * wy1) + tap(x0 + 1, y0 + 1, wx1 * wy1))
    return kxy.astype(np.float32), kpscore.astype(np.float32), dispersity.astype(np.float32)


def kernel(scores_map):
    scores_map = np.ascontiguousarray(np.asarray(scores_map, dtype=np.float32))
    b = scores_map.shape[0]
    assert scores_map.shape == (b, 1, H, W)

    # the int16 device domain is exact only for 2^-23-granular scores (what
    # jax.random.uniform produces); anything else falls back to host NMS.
    probe = scores_map[0, 0, :64].astype(np.float64) * (1 << 23)
    granular_ok = bool(np.all(np.abs(probe - np.round(probe)) < 1e-6))

    results = [(None, None)] * b
    if granular_ok:
        if "nc" not in _CACHED:
            _CACHED["nc"] = _build_nms_kernel()
        in_maps = [{"img": scores_map[i, 0]} for i in range(b)]
        res = run_bass_kernel_spmd(_CACHED["nc"], in_maps, core_ids=list(range(b)))
        results = [(res.results[i]["outE"], res.results[i]["outO"]) for i in range(b)]

    kxy = np.empty((b, TOP_K, 2), np.float32)
    ks = np.empty((b, TOP_K), np.float32)
    sd = np.empty((b, TOP_K), np.float32)
    for i in range(b):
        kxy[i], ks[i], sd[i] = _host_tail(scores_map[i, 0], *results[i])
    return kxy, ks, sd


# revision 11
# speedup vs baseline: 1.0571x; 1.0571x over previous
"""DKD keypoint detection on 8 TRN2 NeuronCores.

Pure data parallel: 1 image per core.
Device: exact simple_nms on an int16-quantized score field.
  Scores are multiples of 2^-23 in [0,1). v = 30000 - (1-s)*2^23, clamped
  below at BG for non-candidates (s < T0 ~ 0.995). Comparisons among
  candidates are exact; background ordering below candidates is preserved
  or collapsed, which is provably equivalent for the final top-4096
  (T0 is far below the 4096th surviving score on any plausible input;
  survivor count is asserted host-side). 3-round NMS (maxpool / dilate /
  equality) runs densely in int16 with an even/odd column parity split.
  Output: post-NMS masked field (int16 planes).
Host: extract survivors (>4096 guaranteed), exact top-4096 ordering by
  (score desc, index asc) replicating jax.lax.top_k tie-breaks, then the
  tiny 4096x25 softmax / dispersity / bilinear tail in float32.
"""
import numpy as np

import concourse.bass as bass
import concourse.bacc as bacc
import concourse.mybir as mybir
from concourse.tile import TileContext
from concourse.bass_utils import run_bass_kernel_spmd

dt = mybir.dt
Alu = mybir.AluOpType
Act = mybir.ActivationFunctionType

H = W = 1024
P = 128            # partitions
RPP = 8            # image rows per partition
HR = 2             # halo rows each side
TR = RPP + 2 * HR  # tile rows = 12
WE = W // 2        # 512 per parity
WP = WE + 4        # 2 pad cols each side (interior at [2, 514))
BG = -28800        # background / suppressed value
M0 = 41944         # (1 - T0) * 2^23  -> T0 = 1 - 41944*2^-23 ~= 0.995
TOP_K = 4096
RADIUS = 2
KSIZE = 5
KK = 25
TEMPERATURE = 0.1

_CACHED = {}

LO, HI = 2, 514            # interior column range
RS, RE = HR, HR + RPP      # payload row range



def _tt2(nc, out, in0, in1, op, gfrac):
    """One logical tensor_tensor (gpsimd TensorTensor fails walrus codegen on
    this toolchain, so no row-split offload — single DVE op)."""
    nc.vector.tensor_tensor(out=out, in0=in0, in1=in1, op=op)


def _maxpool5(nc, pool, fE, fO, outdt, taga, tagb):
    """5x5 separable max pool of parity pair (fE, fO) -> (oE, oO).

    Inputs need valid halo rows and pad cols. The x-stage runs on all TR
    rows (so y-stage needs no halo refresh); y-stage writes payload rows.
    """
    i16 = dt.int16
    mx = pool.tile([P, TR, WP], i16, tag="mp_mx")
    _tt2(nc, mx[:, :, 1:515], fE[:, :, 1:515], fO[:, :, 1:515], Alu.max, 0.2)
    # px[c] = max(mx[c], mx[c+1]) — shared pair-max for both parity outputs
    px = pool.tile([P, TR, WP], i16, tag="mp_px")
    _tt2(nc, px[:, :, 1:514], mx[:, :, 1:514], mx[:, :, 2:515], Alu.max, 0.33)
    xpE = pool.tile([P, TR, WP], i16, tag="mp_xpE")
    xpO = pool.tile([P, TR, WP], i16, tag="mp_xpO")
    # E col c (x = 2(c-2)): window = max(px[c-1], fE[c+1])
    _tt2(nc, xpE[:, :, LO:HI], px[:, :, LO - 1:HI - 1],
         fE[:, :, LO + 1:HI + 1], Alu.max, 0.33)
    # O col c: window = max(px[c], fO[c-1])
    _tt2(nc, xpO[:, :, LO:HI], px[:, :, LO:HI],
         fO[:, :, LO - 1:HI - 1], Alu.max, 0.33)

    outs = []
    for xp, tag, m3tag in ((xpE, taga, "mp_mx"), (xpO, tagb, "mp_px")):
        m3 = pool.tile([P, TR, WP], i16, tag=m3tag)   # reuse dead slot
        _tt2(nc, m3[:, 1:TR - 1, LO:HI], xp[:, 0:TR - 2, LO:HI],
             xp[:, 2:TR, LO:HI], Alu.max, 0.2)
        _tt2(nc, m3[:, 1:TR - 1, LO:HI], m3[:, 1:TR - 1, LO:HI],
             xp[:, 1:TR - 1, LO:HI], Alu.max, 0.2)
        o = pool.tile([P, TR, WP], outdt, tag=tag)
        _tt2(nc, o[:, RS:RE, LO:HI], m3[:, RS - 1:RE - 1, LO:HI],
             m3[:, RS + 1:RE + 1, LO:HI], Alu.max, 0.2)
        outs.append(o)
    return outs


def _halo(nc, f, padcst):
    """Fill pad cols + halo rows of [P, TR, WP] field (payload valid).

    padcst: [P, HR * WP] tile holding the pad value (engine ops cannot start
    at partition 127, so outer halo rows are filled via DMA from it).
    """
    nc.vector.tensor_copy(f[:, RS:RE, 0:LO],
                          padcst[:, 0:1].to_broadcast([P, RPP, LO]))
    nc.vector.tensor_copy(f[:, RS:RE, HI:WP],
                          padcst[:, 0:1].to_broadcast([P, RPP, WP - HI]))
    nc.sync.dma_start(f[1:P, 0:HR, :], f[0:P - 1, RPP:RPP + HR, :])
    nc.sync.dma_start(f[0:P - 1, RE:TR, :], f[1:P, HR:2 * HR, :])
    nc.sync.dma_start(f[0:1, 0:HR, :],
                      padcst[0:1, :].rearrange("p (r w) -> p r w", r=HR))
    nc.sync.dma_start(f[P - 1:P, RE:TR, :],
                      padcst[0:1, :].rearrange("p (r w) -> p r w", r=HR))


def _build_nms_kernel():
    nc = bacc.Bacc(None, target_bir_lowering=False)
    img = nc.dram_tensor("img", [H, W], dt.float32, kind="ExternalInput")
    outE = nc.dram_tensor("outE", [H, WE], dt.int16, kind="ExternalOutput")
    outO = nc.dram_tensor("outO", [H, WE], dt.int16, kind="ExternalOutput")
    imgr = img.rearrange("(p r) w -> p r w", p=P)

    i16 = dt.int16
    with TileContext(nc) as tc:
        with tc.tile_pool(name="sb", bufs=1) as pool, \
             tc.tile_pool(name="ld", bufs=2) as ldp:
            bgc = pool.tile([P, HR * WP], i16, tag="bgc")
            nc.vector.memset(bgc[:], BG)
            zc = pool.tile([P, HR * WP], i16, tag="zc")
            nc.vector.memset(zc[:], 0)
            vE = pool.tile([P, TR, WP], i16, tag="vE")
            vO = pool.tile([P, TR, WP], i16, tag="vO")
            # ---- chunked load + v-compute ----
            for r in range(RPP):
                stage = ldp.tile([P, 1, W], dt.float32, tag="stage")
                nc.sync.dma_start(stage[:], imgr[:, r:r + 1, :])
                tf = pool.tile([P, 1, W], dt.float32, tag="tf")
                nc.scalar.activation(tf[:], stage[:], Act.Copy,
                                     bias=float(30000 - (1 << 23)),
                                     scale=float(1 << 23))
                tv = tf[:].rearrange("p r (w two) -> p r two w", two=2)
                nc.vector.tensor_scalar(vE[:, RS + r:RS + r + 1, LO:HI],
                                        tv[:, :, 0, :], float(BG),
                                        scalar2=None, op0=Alu.max)
                nc.vector.tensor_scalar(vO[:, RS + r:RS + r + 1, LO:HI],
                                        tv[:, :, 1, :], float(BG),
                                        scalar2=None, op0=Alu.max)
            _halo(nc, vE, bgc)
            _halo(nc, vO, bgc)

            # ---- round 0 ----
            p1E, p1O = _maxpool5(nc, pool, vE, vO, i16, "pAE", "pAO")
            m0E = pool.tile([P, TR, WP], i16, tag="m0E")
            m0O = pool.tile([P, TR, WP], i16, tag="m0O")
            for v_, p_, m_ in ((vE, p1E, m0E), (vO, p1O, m0O)):
                _tt2(nc, m_[:, RS:RE, LO:HI], v_[:, RS:RE, LO:HI],
                     p_[:, RS:RE, LO:HI], Alu.is_equal, 0.2)
            _halo(nc, m0E, zc)
            _halo(nc, m0O, zc)

            # ---- round 1 ----
            s1E, s1O = _maxpool5(nc, pool, m0E, m0O, dt.int8, "sBE", "sBO")
            svE = pool.tile([P, TR, WP], i16, tag="svE")
            svO = pool.tile([P, TR, WP], i16, tag="svO")
            for v_, s_, sv_ in ((vE, s1E, svE), (vO, s1O, svO)):
                nc.vector.tensor_copy(sv_[:, RS:RE, LO:HI], v_[:, RS:RE, LO:HI])
                nc.vector.copy_predicated(sv_[:, RS:RE, LO:HI], s_[:, RS:RE, LO:HI],
                                          bgc[:, 0:1].to_broadcast([P, RPP, WE]))
            _halo(nc, svE, bgc)
            _halo(nc, svO, bgc)
            p3E, p3O = _maxpool5(nc, pool, svE, svO, i16, "pAE", "pAO")
            m1E = pool.tile([P, TR, WP], i16, tag="m1E")
            m1O = pool.tile([P, TR, WP], i16, tag="m1O")
            for sv_, p_, s_, m0_, m1_ in ((svE, p3E, s1E, m0E, m1E),
                                          (svO, p3O, s1O, m0O, m1O)):
                eq = pool.tile([P, TR, WP], dt.int8, tag="eq")
                _tt2(nc, eq[:, RS:RE, LO:HI], sv_[:, RS:RE, LO:HI],
                     p_[:, RS:RE, LO:HI], Alu.is_equal, 0.2)
                # new1 = eq & !supp1 ; mask1 = max(mask0, new1)
                nc.vector.scalar_tensor_tensor(out=eq[:, RS:RE, LO:HI],
                                               in0=s_[:, RS:RE, LO:HI], scalar=0.0,
                                               in1=eq[:, RS:RE, LO:HI],
                                               op0=Alu.is_equal, op1=Alu.mult)
                _tt2(nc, m1_[:, RS:RE, LO:HI], m0_[:, RS:RE, LO:HI],
                     eq[:, RS:RE, LO:HI], Alu.max, 0.2)
            _halo(nc, m1E, zc)
            _halo(nc, m1O, zc)

            # ---- round 2 ----
            s2E, s2O = _maxpool5(nc, pool, m1E, m1O, dt.int8, "sBE", "sBO")
            for v_, s_, sv_ in ((vE, s2E, svE), (vO, s2O, svO)):
                nc.vector.tensor_copy(sv_[:, RS:RE, LO:HI], v_[:, RS:RE, LO:HI])
                nc.vector.copy_predicated(sv_[:, RS:RE, LO:HI], s_[:, RS:RE, LO:HI],
                                          bgc[:, 0:1].to_broadcast([P, RPP, WE]))
            _halo(nc, svE, bgc)
            _halo(nc, svO, bgc)
            p5E, p5O = _maxpool5(nc, pool, svE, svO, i16, "pAE", "pAO")

            for v_, sv_, p_, s_, m1_, out_ in (
                    (vE, svE, p5E, s2E, m1E, outE),
                    (vO, svO, p5O, s2O, m1O, outO)):
                eq = pool.tile([P, TR, WP], dt.int8, tag="eq")
                _tt2(nc, eq[:, RS:RE, LO:HI], sv_[:, RS:RE, LO:HI],
                     p_[:, RS:RE, LO:HI], Alu.is_equal, 0.2)
                nc.vector.scalar_tensor_tensor(out=eq[:, RS:RE, LO:HI],
                                               in0=s_[:, RS:RE, LO:HI], scalar=0.0,
                                               in1=eq[:, RS:RE, LO:HI],
                                               op0=Alu.is_equal, op1=Alu.mult)
                _tt2(nc, eq[:, RS:RE, LO:HI], m1_[:, RS:RE, LO:HI],
                     eq[:, RS:RE, LO:HI], Alu.max, 0.2)
                svf = pool.tile([P, TR, WP], i16, tag="mp_xpE")  # reuse slot
                nc.vector.memset(svf[:, RS:RE, LO:HI], BG)
                nc.vector.copy_predicated(svf[:, RS:RE, LO:HI], eq[:, RS:RE, LO:HI],
                                          v_[:, RS:RE, LO:HI])
                nc.sync.dma_start(out_.rearrange("(p r) w -> p r w", p=P)[:, :, :],
                                  svf[:, RS:RE, LO:HI])
    nc.compile()
    return nc


def _numpy_nms(img):
    """Full-precision numpy replica of the reference NMS (fallback path)."""
    def maxpool(x):
        p = np.pad(x, RADIUS, mode="constant", constant_values=-np.inf)
        out = np.full_like(x, -np.inf)
        for dy in range(KSIZE):
            for dx in range(KSIZE):
                out = np.maximum(out, p[dy:dy + H, dx:dx + W])
        return out

    zeros = np.zeros_like(img)
    max_mask = img == maxpool(img)
    for _ in range(2):
        supp = maxpool(max_mask.astype(np.float32)) > 0
        ss = np.where(supp, zeros, img)
        new = ss == maxpool(ss)
        max_mask = max_mask | (new & ~supp)
    nms = np.where(max_mask, img, 0.0)
    nms[:2] = nms[-2:] = 0.0
    nms[:, :2] = nms[:, -2:] = 0.0
    flat = nms.reshape(-1)
    idx = np.argsort(-flat, kind="stable")[:TOP_K]
    return idx // W, idx % W


def _host_tail(img, vE, vO):
    """Exact top-4096 + keypoint refinement from the device NMS field."""
    if vE is None:
        ys, xs = _numpy_nms(img)
    else:
        sv = np.empty((H, W), np.int16)
        sv[:, 0::2] = vE
        sv[:, 1::2] = vO
        sv[:2] = BG; sv[-2:] = BG; sv[:, :2] = BG; sv[:, -2:] = BG
        ys, xs = np.nonzero(sv > BG)
        if len(ys) < TOP_K:
            # T0 margin insufficient for this input: recompute exactly on host.
            ys, xs = _numpy_nms(img)
        else:
            m = 30000 - sv[ys, xs].astype(np.int64)
            idx = ys.astype(np.int64) * W + xs
            order = np.lexsort((idx, m))[:TOP_K]
            ys, xs = ys[order], xs[order]

    padded = np.pad(img, ((RADIUS, RADIUS), (RADIUS, RADIUS)))
    patches = np.empty((TOP_K, KSIZE, KSIZE), np.float32)
    for dy in range(KSIZE):
        for dx in range(KSIZE):
            patches[:, dy, dx] = padded[ys + dy, xs + dx]
    patches = patches.reshape(TOP_K, KK)

    lin = np.linspace(-RADIUS, RADIUS, KSIZE, dtype=np.float32)
    gx, gy = np.meshgrid(lin, lin)
    hw_grid = np.stack([gx, gy], axis=-1).reshape(-1, 2).astype(np.float32)

    max_v = patches.max(axis=-1, keepdims=True)
    x_exp = np.exp((patches - max_v) / np.float32(TEMPERATURE), dtype=np.float32)
    x_exp_sum = x_exp.sum(axis=-1, keepdims=True, dtype=np.float32)
    xy_res = (x_exp @ hw_grid) / x_exp_sum
    diff = (hw_grid[None, :, :] - xy_res[:, None, :]) / np.float32(RADIUS)
    hw_dist2 = (diff * diff).sum(-1, dtype=np.float32)
    dispersity = (x_exp * hw_dist2).sum(-1, dtype=np.float32) / x_exp_sum[:, 0]

    kp_nms = np.stack([xs, ys], axis=-1).astype(np.float32)
    wh = np.array([W - 1, H - 1], np.float32)
    kxy = (kp_nms + xy_res) / wh * np.float32(2.0) - np.float32(1.0)

    px = ((kxy[:, 0] + 1.0) * 0.5 * (W - 1)).astype(np.float32)
    py = ((kxy[:, 1] + 1.0) * 0.5 * (H - 1)).astype(np.float32)
    x0 = np.floor(px); y0 = np.floor(py)
    wx1 = (px - x0).astype(np.float32); wx0 = np.float32(1.0) - wx1
    wy1 = (py - y0).astype(np.float32); wy0 = np.float32(1.0) - wy1

    def tap(xi, yi, wgt):
        valid = (xi >= 0) & (xi < W) & (yi >= 0) & (yi < H)
        xc = np.clip(xi, 0, W - 1).astype(np.int64)
        yc = np.clip(yi, 0, H - 1).astype(np.int64)
        return np.where(valid, img[yc, xc], 0.0).astype(np.float32) * wgt

    kpscore = (tap(x0, y0, wx0 * wy0) + tap(x0 + 1, y0, wx1 * wy0)
               + tap(x0, y0 + 1, wx0 * wy1) + tap(x0 + 1, y0 + 1, wx1 * wy1))
    return kxy.astype(np.float32), kpscore.astype(np.float32), dispersity.astype(np.float32)


def kernel(scores_map):
    scores_map = np.ascontiguousarray(np.asarray(scores_map, dtype=np.float32))
    b = scores_map.shape[0]
    assert scores_map.shape == (b, 1, H, W)

    # the int16 device domain is exact only for 2^-23-granular scores (what
    # jax.random.uniform produces); anything else falls back to host NMS.
    probe = scores_map[0, 0, :64].astype(np.float64) * (1 << 23)
    granular_ok = bool(np.all(np.abs(probe - np.round(probe)) < 1e-6))

    results = [(None, None)] * b
    if granular_ok:
        if "nc" not in _CACHED:
            _CACHED["nc"] = _build_nms_kernel()
        in_maps = [{"img": scores_map[i, 0]} for i in range(b)]
        res = run_bass_kernel_spmd(_CACHED["nc"], in_maps, core_ids=list(range(b)))
        results = [(res.results[i]["outE"], res.results[i]["outO"]) for i in range(b)]

    kxy = np.empty((b, TOP_K, 2), np.float32)
    ks = np.empty((b, TOP_K), np.float32)
    sd = np.empty((b, TOP_K), np.float32)
    for i in range(b):
        kxy[i], ks[i], sd[i] = _host_tail(scores_map[i, 0], *results[i])
    return kxy, ks, sd


# revision 12
# speedup vs baseline: 1.0925x; 1.0335x over previous
"""DKD keypoint detection on 8 TRN2 NeuronCores.

Pure data parallel: 1 image per core.
Device: exact simple_nms on an int16-quantized score field.
  Scores are multiples of 2^-23 in [0,1). v = 30000 - (1-s)*2^23, clamped
  below at BG for non-candidates (s < T0 ~ 0.995). Comparisons among
  candidates are exact; background ordering below candidates is preserved
  or collapsed, which is provably equivalent for the final top-4096
  (T0 is far below the 4096th surviving score on any plausible input;
  survivor count is asserted host-side). 3-round NMS (maxpool / dilate /
  equality) runs densely in int16 with an even/odd column parity split.
  Output: post-NMS masked field (int16 planes).
Host: extract survivors (>4096 guaranteed), exact top-4096 ordering by
  (score desc, index asc) replicating jax.lax.top_k tie-breaks, then the
  tiny 4096x25 softmax / dispersity / bilinear tail in float32.
"""
import numpy as np

import concourse.bass as bass
import concourse.bacc as bacc
import concourse.mybir as mybir
from concourse.tile import TileContext
from concourse.bass_utils import run_bass_kernel_spmd

dt = mybir.dt
Alu = mybir.AluOpType
Act = mybir.ActivationFunctionType

H = W = 1024
P = 128            # partitions
RPP = 8            # image rows per partition
HR = 2             # halo rows each side
TR = RPP + 2 * HR  # tile rows = 12
WE = W // 2        # 512 per parity
WP = WE + 4        # 2 pad cols each side (interior at [2, 514))
BG = -28800        # background / suppressed value
M0 = 41944         # (1 - T0) * 2^23  -> T0 = 1 - 41944*2^-23 ~= 0.995
TOP_K = 4096
RADIUS = 2
KSIZE = 5
KK = 25
TEMPERATURE = 0.1

_CACHED = {}

LO, HI = 2, 514            # interior column range
RS, RE = HR, HR + RPP      # payload row range



def _tt2(nc, out, in0, in1, op, gfrac):
    """One logical tensor_tensor (gpsimd TensorTensor fails walrus codegen on
    this toolchain, so no row-split offload — single DVE op)."""
    nc.vector.tensor_tensor(out=out, in0=in0, in1=in1, op=op)


def _maxpool5(nc, pool, fE, fO, outdt, taga, tagb):
    """5x5 separable max pool of parity pair (fE, fO) -> (oE, oO).

    Inputs need valid halo rows and pad cols. The x-stage runs on all TR
    rows (so y-stage needs no halo refresh); y-stage writes payload rows.
    """
    i16 = dt.int16
    mx = pool.tile([P, TR, WP], i16, tag="mp_mx")
    _tt2(nc, mx[:, :, 1:515], fE[:, :, 1:515], fO[:, :, 1:515], Alu.max, 0.2)
    # px[c] = max(mx[c], mx[c+1]) — shared pair-max for both parity outputs
    px = pool.tile([P, TR, WP], i16, tag="mp_px")
    _tt2(nc, px[:, :, 1:514], mx[:, :, 1:514], mx[:, :, 2:515], Alu.max, 0.33)
    xpE = pool.tile([P, TR, WP], i16, tag="mp_xpE")
    xpO = pool.tile([P, TR, WP], i16, tag="mp_xpO")
    # E col c (x = 2(c-2)): window = max(px[c-1], fE[c+1])
    _tt2(nc, xpE[:, :, LO:HI], px[:, :, LO - 1:HI - 1],
         fE[:, :, LO + 1:HI + 1], Alu.max, 0.33)
    # O col c: window = max(px[c], fO[c-1])
    _tt2(nc, xpO[:, :, LO:HI], px[:, :, LO:HI],
         fO[:, :, LO - 1:HI - 1], Alu.max, 0.33)

    outs = []
    for xp, tag, m3tag in ((xpE, taga, "mp_mx"), (xpO, tagb, "mp_px")):
        m3 = pool.tile([P, TR, WP], i16, tag=m3tag)   # reuse dead slot
        _tt2(nc, m3[:, 1:TR - 1, LO:HI], xp[:, 0:TR - 2, LO:HI],
             xp[:, 2:TR, LO:HI], Alu.max, 0.2)
        _tt2(nc, m3[:, 1:TR - 1, LO:HI], m3[:, 1:TR - 1, LO:HI],
             xp[:, 1:TR - 1, LO:HI], Alu.max, 0.2)
        o = pool.tile([P, TR, WP], outdt, tag=tag)
        _tt2(nc, o[:, RS:RE, LO:HI], m3[:, RS - 1:RE - 1, LO:HI],
             m3[:, RS + 1:RE + 1, LO:HI], Alu.max, 0.2)
        outs.append(o)
    return outs


def _halo(nc, f, padcst):
    """Fill pad cols + halo rows of [P, TR, WP] field (payload valid).

    padcst: [P, HR * WP] tile holding the pad value (engine ops cannot start
    at partition 127, so outer halo rows are filled via DMA from it).
    """
    nc.vector.tensor_copy(f[:, RS:RE, 0:LO],
                          padcst[:, 0:1].to_broadcast([P, RPP, LO]))
    nc.vector.tensor_copy(f[:, RS:RE, HI:WP],
                          padcst[:, 0:1].to_broadcast([P, RPP, WP - HI]))
    nc.sync.dma_start(f[1:P, 0:HR, :], f[0:P - 1, RPP:RPP + HR, :])
    nc.sync.dma_start(f[0:P - 1, RE:TR, :], f[1:P, HR:2 * HR, :])
    nc.sync.dma_start(f[0:1, 0:HR, :],
                      padcst[0:1, :].rearrange("p (r w) -> p r w", r=HR))
    nc.sync.dma_start(f[P - 1:P, RE:TR, :],
                      padcst[0:1, :].rearrange("p (r w) -> p r w", r=HR))


def _build_nms_kernel():
    nc = bacc.Bacc(None, target_bir_lowering=False)
    img = nc.dram_tensor("img", [H, W], dt.float32, kind="ExternalInput")
    outE = nc.dram_tensor("outE", [H, WE], dt.int16, kind="ExternalOutput")
    outO = nc.dram_tensor("outO", [H, WE], dt.int16, kind="ExternalOutput")
    imgr = img.rearrange("(p r) w -> p r w", p=P)

    i16 = dt.int16
    with TileContext(nc) as tc:
        with tc.tile_pool(name="sb", bufs=1) as pool, \
             tc.tile_pool(name="ld", bufs=2) as ldp:
            bgc = pool.tile([P, HR * WP], i16, tag="bgc")
            nc.vector.memset(bgc[:], BG)
            zc = pool.tile([P, HR * WP], i16, tag="zc")
            nc.vector.memset(zc[:], 0)
            vE = pool.tile([P, TR, WP], i16, tag="vE")
            vO = pool.tile([P, TR, WP], i16, tag="vO")
            # ---- chunked load + v-compute ----
            for r in range(RPP):
                stage = ldp.tile([P, 1, W], dt.float32, tag="stage")
                nc.sync.dma_start(stage[:], imgr[:, r:r + 1, :])
                tf = pool.tile([P, 1, W], dt.float32, tag="tf")
                nc.scalar.activation(tf[:], stage[:], Act.Copy,
                                     bias=float(30000 - (1 << 23)),
                                     scale=float(1 << 23))
                tv = tf[:].rearrange("p r (w two) -> p r two w", two=2)
                nc.vector.tensor_scalar(vE[:, RS + r:RS + r + 1, LO:HI],
                                        tv[:, :, 0, :], float(BG),
                                        scalar2=None, op0=Alu.max)
                nc.vector.tensor_scalar(vO[:, RS + r:RS + r + 1, LO:HI],
                                        tv[:, :, 1, :], float(BG),
                                        scalar2=None, op0=Alu.max)
            _halo(nc, vE, bgc)
            _halo(nc, vO, bgc)

            # ---- round 0 ----
            p1E, p1O = _maxpool5(nc, pool, vE, vO, i16, "pAE", "pAO")
            m0E = pool.tile([P, TR, WP], i16, tag="m0E")
            m0O = pool.tile([P, TR, WP], i16, tag="m0O")
            for v_, p_, m_ in ((vE, p1E, m0E), (vO, p1O, m0O)):
                _tt2(nc, m_[:, RS:RE, LO:HI], v_[:, RS:RE, LO:HI],
                     p_[:, RS:RE, LO:HI], Alu.is_equal, 0.2)
            _halo(nc, m0E, zc)
            _halo(nc, m0O, zc)

            # ---- round 1 ----
            s1E, s1O = _maxpool5(nc, pool, m0E, m0O, dt.int8, "sBE", "sBO")
            svE = pool.tile([P, TR, WP], i16, tag="svE")
            svO = pool.tile([P, TR, WP], i16, tag="svO")
            for v_, s_, sv_ in ((vE, s1E, svE), (vO, s1O, svO)):
                t_ = pool.tile([P, TR, WP], i16, tag="mp_xpO")  # reuse dead slot
                nc.vector.tensor_scalar(t_[:, RS:RE, LO:HI], s_[:, RS:RE, LO:HI],
                                        -65535.0, scalar2=32767.0,
                                        op0=Alu.mult, op1=Alu.add)
                nc.vector.tensor_tensor(out=sv_[:, RS:RE, LO:HI],
                                        in0=v_[:, RS:RE, LO:HI],
                                        in1=t_[:, RS:RE, LO:HI], op=Alu.min)
            _halo(nc, svE, bgc)
            _halo(nc, svO, bgc)
            p3E, p3O = _maxpool5(nc, pool, svE, svO, i16, "pAE", "pAO")
            m1E = pool.tile([P, TR, WP], i16, tag="m1E")
            m1O = pool.tile([P, TR, WP], i16, tag="m1O")
            for sv_, p_, s_, m0_, m1_ in ((svE, p3E, s1E, m0E, m1E),
                                          (svO, p3O, s1O, m0O, m1O)):
                eq = pool.tile([P, TR, WP], dt.int8, tag="eq")
                _tt2(nc, eq[:, RS:RE, LO:HI], sv_[:, RS:RE, LO:HI],
                     p_[:, RS:RE, LO:HI], Alu.is_equal, 0.2)
                # new1 = eq & !supp1 ; mask1 = max(mask0, new1)
                nc.vector.scalar_tensor_tensor(out=eq[:, RS:RE, LO:HI],
                                               in0=s_[:, RS:RE, LO:HI], scalar=0.0,
                                               in1=eq[:, RS:RE, LO:HI],
                                               op0=Alu.is_equal, op1=Alu.mult)
                _tt2(nc, m1_[:, RS:RE, LO:HI], m0_[:, RS:RE, LO:HI],
                     eq[:, RS:RE, LO:HI], Alu.max, 0.2)
            _halo(nc, m1E, zc)
            _halo(nc, m1O, zc)

            # ---- round 2 ----
            s2E, s2O = _maxpool5(nc, pool, m1E, m1O, dt.int8, "sBE", "sBO")
            for v_, s_, sv_ in ((vE, s2E, svE), (vO, s2O, svO)):
                t_ = pool.tile([P, TR, WP], i16, tag="mp_xpO")  # reuse dead slot
                nc.vector.tensor_scalar(t_[:, RS:RE, LO:HI], s_[:, RS:RE, LO:HI],
                                        -65535.0, scalar2=32767.0,
                                        op0=Alu.mult, op1=Alu.add)
                nc.vector.tensor_tensor(out=sv_[:, RS:RE, LO:HI],
                                        in0=v_[:, RS:RE, LO:HI],
                                        in1=t_[:, RS:RE, LO:HI], op=Alu.min)
            _halo(nc, svE, bgc)
            _halo(nc, svO, bgc)
            p5E, p5O = _maxpool5(nc, pool, svE, svO, i16, "pAE", "pAO")

            for v_, sv_, p_, s_, m1_, out_ in (
                    (vE, svE, p5E, s2E, m1E, outE),
                    (vO, svO, p5O, s2O, m1O, outO)):
                eq = pool.tile([P, TR, WP], dt.int8, tag="eq")
                _tt2(nc, eq[:, RS:RE, LO:HI], sv_[:, RS:RE, LO:HI],
                     p_[:, RS:RE, LO:HI], Alu.is_equal, 0.2)
                nc.vector.scalar_tensor_tensor(out=eq[:, RS:RE, LO:HI],
                                               in0=s_[:, RS:RE, LO:HI], scalar=0.0,
                                               in1=eq[:, RS:RE, LO:HI],
                                               op0=Alu.is_equal, op1=Alu.mult)
                _tt2(nc, eq[:, RS:RE, LO:HI], m1_[:, RS:RE, LO:HI],
                     eq[:, RS:RE, LO:HI], Alu.max, 0.2)
                svf = pool.tile([P, TR, WP], i16, tag="mp_xpE")  # reuse slot
                nc.vector.tensor_scalar(svf[:, RS:RE, LO:HI], eq[:, RS:RE, LO:HI],
                                        65535.0, scalar2=-32768.0,
                                        op0=Alu.mult, op1=Alu.add)
                nc.vector.tensor_tensor(out=svf[:, RS:RE, LO:HI],
                                        in0=v_[:, RS:RE, LO:HI],
                                        in1=svf[:, RS:RE, LO:HI], op=Alu.min)
                nc.sync.dma_start(out_.rearrange("(p r) w -> p r w", p=P)[:, :, :],
                                  svf[:, RS:RE, LO:HI])
    nc.compile()
    return nc


def _numpy_nms(img):
    """Full-precision numpy replica of the reference NMS (fallback path)."""
    def maxpool(x):
        p = np.pad(x, RADIUS, mode="constant", constant_values=-np.inf)
        out = np.full_like(x, -np.inf)
        for dy in range(KSIZE):
            for dx in range(KSIZE):
                out = np.maximum(out, p[dy:dy + H, dx:dx + W])
        return out

    zeros = np.zeros_like(img)
    max_mask = img == maxpool(img)
    for _ in range(2):
        supp = maxpool(max_mask.astype(np.float32)) > 0
        ss = np.where(supp, zeros, img)
        new = ss == maxpool(ss)
        max_mask = max_mask | (new & ~supp)
    nms = np.where(max_mask, img, 0.0)
    nms[:2] = nms[-2:] = 0.0
    nms[:, :2] = nms[:, -2:] = 0.0
    flat = nms.reshape(-1)
    idx = np.argsort(-flat, kind="stable")[:TOP_K]
    return idx // W, idx % W


def _host_tail(img, vE, vO):
    """Exact top-4096 + keypoint refinement from the device NMS field."""
    if vE is None:
        ys, xs = _numpy_nms(img)
    else:
        sv = np.empty((H, W), np.int16)
        sv[:, 0::2] = vE
        sv[:, 1::2] = vO
        sv[:2] = BG; sv[-2:] = BG; sv[:, :2] = BG; sv[:, -2:] = BG
        ys, xs = np.nonzero(sv > BG)
        if len(ys) < TOP_K:
            # T0 margin insufficient for this input: recompute exactly on host.
            ys, xs = _numpy_nms(img)
        else:
            m = 30000 - sv[ys, xs].astype(np.int64)
            idx = ys.astype(np.int64) * W + xs
            order = np.lexsort((idx, m))[:TOP_K]
            ys, xs = ys[order], xs[order]

    padded = np.pad(img, ((RADIUS, RADIUS), (RADIUS, RADIUS)))
    patches = np.empty((TOP_K, KSIZE, KSIZE), np.float32)
    for dy in range(KSIZE):
        for dx in range(KSIZE):
            patches[:, dy, dx] = padded[ys + dy, xs + dx]
    patches = patches.reshape(TOP_K, KK)

    lin = np.linspace(-RADIUS, RADIUS, KSIZE, dtype=np.float32)
    gx, gy = np.meshgrid(lin, lin)
    hw_grid = np.stack([gx, gy], axis=-1).reshape(-1, 2).astype(np.float32)

    max_v = patches.max(axis=-1, keepdims=True)
    x_exp = np.exp((patches - max_v) / np.float32(TEMPERATURE), dtype=np.float32)
    x_exp_sum = x_exp.sum(axis=-1, keepdims=True, dtype=np.float32)
    xy_res = (x_exp @ hw_grid) / x_exp_sum
    diff = (hw_grid[None, :, :] - xy_res[:, None, :]) / np.float32(RADIUS)
    hw_dist2 = (diff * diff).sum(-1, dtype=np.float32)
    dispersity = (x_exp * hw_dist2).sum(-1, dtype=np.float32) / x_exp_sum[:, 0]

    kp_nms = np.stack([xs, ys], axis=-1).astype(np.float32)
    wh = np.array([W - 1, H - 1], np.float32)
    kxy = (kp_nms + xy_res) / wh * np.float32(2.0) - np.float32(1.0)

    px = ((kxy[:, 0] + 1.0) * 0.5 * (W - 1)).astype(np.float32)
    py = ((kxy[:, 1] + 1.0) * 0.5 * (H - 1)).astype(np.float32)
    x0 = np.floor(px); y0 = np.floor(py)
    wx1 = (px - x0).astype(np.float32); wx0 = np.float32(1.0) - wx1
    wy1 = (py - y0).astype(np.float32); wy0 = np.float32(1.0) - wy1

    def tap(xi, yi, wgt):
        valid = (xi >= 0) & (xi < W) & (yi >= 0) & (yi < H)
        xc = np.clip(xi, 0, W - 1).astype(np.int64)
        yc = np.clip(yi, 0, H - 1).astype(np.int64)
        return np.where(valid, img[yc, xc], 0.0).astype(np.float32) * wgt

    kpscore = (tap(x0, y0, wx0 * wy0) + tap(x0 + 1, y0, wx1 * wy0)
               + tap(x0, y0 + 1, wx0 * wy1) + tap(x0 + 1, y0 + 1, wx1 * wy1))
    return kxy.astype(np.float32), kpscore.astype(np.float32), dispersity.astype(np.float32)


def kernel(scores_map):
    scores_map = np.ascontiguousarray(np.asarray(scores_map, dtype=np.float32))
    b = scores_map.shape[0]
    assert scores_map.shape == (b, 1, H, W)

    # the int16 device domain is exact only for 2^-23-granular scores (what
    # jax.random.uniform produces); anything else falls back to host NMS.
    probe = scores_map[0, 0, :64].astype(np.float64) * (1 << 23)
    granular_ok = bool(np.all(np.abs(probe - np.round(probe)) < 1e-6))

    results = [(None, None)] * b
    if granular_ok:
        if "nc" not in _CACHED:
            _CACHED["nc"] = _build_nms_kernel()
        in_maps = [{"img": scores_map[i, 0]} for i in range(b)]
        res = run_bass_kernel_spmd(_CACHED["nc"], in_maps, core_ids=list(range(b)))
        results = [(res.results[i]["outE"], res.results[i]["outO"]) for i in range(b)]

    kxy = np.empty((b, TOP_K, 2), np.float32)
    ks = np.empty((b, TOP_K), np.float32)
    sd = np.empty((b, TOP_K), np.float32)
    for i in range(b):
        kxy[i], ks[i], sd[i] = _host_tail(scores_map[i, 0], *results[i])
    return kxy, ks, sd
